# revision 59
# baseline (speedup 1.0000x reference)
"""Cosine-similarity kernel (x[16384,512] vs weights[4096,512] -> [16384,4096])
on 8 Trainium2 NeuronCores, sharded as a 4x2 grid (4-way batch x 2-way N).

Strategy: cos(x, w) = (x/|x|)·(w/|w|) is a normalized GEMM.  All cheap
O(B*D) prep runs on host: normalize, scale by S, quantize to TRN fp8 e4m3,
and pre-pack transposed k-tile-paired layouts.  The device does only the
O(B*N*D) GEMM as DoubleRow fp8 matmuls (2 k-rows per partition), PSUM
accumulation, fp16 eviction, and big contiguous DMAs.

Precision: the w-side is sent as a two-term fp8 expansion (w8 + s8), so the
device computes x8·(w8+s8) [+ optionally r8·w8 over half of K], keeping
max|err|/absmax(ref) at 1.90e-2 [1.38e-2], under the 2e-2 gate on the fixed
problem seed (device arithmetic reproduces the numpy emulation to ~1e-6).

Layouts: both k-groups are packed into one tensor per operand
([128, 4, rows]: slot = kgroup*2 + pair) so each input DMA piece is >=728ns
and the SP-sequencer issue cadence (565ns/DMA) never starves the DMA pipe.
"""
import numpy as np
import ml_dtypes

B, D, N = 16384, 512, 4096
GR, GC = 4, 2             # grid: GR batch shards x GC column shards
NCORES = GR * GC
BS = B // GR              # 4096 batch rows per core
NW = N // GC              # 2048 output cols per core
MT = BS // 128            # 32 m-tiles
NB = NW // 512            # 4 psum column blocks
NCHUNK = 2                # n-column chunks of 1024 for DMA/compute overlap
SCALE = 128.0             # fp8 dynamic-range scale; out = psum / SCALE^2
R_COMP = 0                # 1: add r8·w8 over K/2 (rel~1.38e-2), 0: rel~1.90e-2
WARMUP = 63               # PE p-state warmup matmuls during input DMA window

E4 = ml_dtypes.float8_e4m3  # IEEE-style e4m3, max normal 240 == TRN FP8_EXP4

_cached = {}


def _build():
    import concourse.bass as bass
    import concourse.mybir as mybir
    import concourse.tile as tile
    from concourse import bacc

    F32, F16, F8 = mybir.dt.float32, mybir.dt.float16, mybir.dt.float8e4
    DR = mybir.MatmulPerfMode.DoubleRow
    CW = NW // NCHUNK     # 1024 cols per chunk

    nc = bacc.Bacc(None, target_bir_lowering=False)
    x8d = nc.dram_tensor("x8", [128, 4, BS], F8, kind="ExternalInput")
    w8d = nc.dram_tensor("w8", [128, 4, NW], F8, kind="ExternalInput")
    s8d = nc.dram_tensor("s8", [128, 4, NW], F8, kind="ExternalInput")
    r8d = (nc.dram_tensor("r8", [128, 2, BS], F8, kind="ExternalInput")
           if R_COMP else None)
    outd = nc.dram_tensor("out", [BS, NW], F16, kind="ExternalOutput")

    with tile.TileContext(nc) as tc:
        with (
            tc.tile_pool(name="ops", bufs=1) as ops,
            tc.tile_pool(name="ostage", bufs=34) as ostage,
            tc.tile_pool(name="mmps", bufs=8, space="PSUM") as mmps,
        ):
            x8t = ops.tile([128, 4, BS], F8, name="x8t")
            w8t = ops.tile([128, 4, NW], F8, name="w8t")
            s8t = ops.tile([128, 4, NW], F8, name="s8t")
            r8t = ops.tile([128, 2, BS], F8, name="r8t") if R_COMP else None
            zpad = ops.tile([128, 2, 128], F8, name="zpad")
            wps = mmps.tile([128, 512], F32, name="wps", tag="pm")

            # Warm up the PE p-state during the input-DMA window: dummy
            # DoubleRow matmuls on a zeroed tile so the real matmuls start at
            # the full 2.4 GHz clock instead of paying the 3us ramp.  The
            # memset is tiny so the PE busy-clock starts early enough that
            # even the first real matmuls run at full speed.
            nc.vector.memset(zpad[:], 0)
            for _ in range(WARMUP):
                nc.tensor.matmul(wps[:, :128], zpad[:], zpad[:],
                                 start=True, stop=True, perf_mode=DR)

            # Input supply matched to PE consumption: x rows 0:512 + w/s
            # chunk-0 cols, then ascending x row ranges, then chunk 1.
            # Every piece is >=728ns of transfer so the 565ns/instr SP
            # sequencer issue rate keeps the DMA pipe saturated.
            h0, h1 = slice(0, 512), slice(512, CW)
            nc.sync.dma_start(x8t[:, :, :640], x8d[:, :, :640])
            nc.sync.dma_start(w8t[:, :, h0], w8d[:, :, h0])
            nc.sync.dma_start(s8t[:, :, h0], s8d[:, :, h0])
            nc.sync.dma_start(x8t[:, :, 640:1024], x8d[:, :, 640:1024])
            nc.sync.dma_start(w8t[:, :, h1], w8d[:, :, h1])
            nc.sync.dma_start(s8t[:, :, h1], s8d[:, :, h1])
            for rs in (slice(1024, 2048), slice(2048, BS)):
                nc.sync.dma_start(x8t[:, :, rs], x8d[:, :, rs])
            if R_COMP:
                nc.sync.dma_start(r8t[:], r8d[:])
            for c in range(1, NCHUNK):
                cs = slice(c * CW, (c + 1) * CW)
                nc.sync.dma_start(w8t[:, :, cs], w8d[:, :, cs])
                nc.sync.dma_start(s8t[:, :, cs], s8d[:, :, cs])

            g0, g1 = slice(0, 2), slice(2, 4)
            ev = 0
            LEAD = 5          # nb0-only lead tiles: saturates PE on 3 DMAs

            def group(pm, ms, ns):
                nc.tensor.matmul(pm[:], x8t[:, g0, ms], w8t[:, g0, ns],
                                 start=True, stop=False, perf_mode=DR)
                nc.tensor.matmul(pm[:], x8t[:, g1, ms], w8t[:, g1, ns],
                                 start=False, stop=False, perf_mode=DR)
                nc.tensor.matmul(pm[:], x8t[:, g0, ms], s8t[:, g0, ns],
                                 start=False, stop=False, perf_mode=DR)
                nc.tensor.matmul(pm[:], x8t[:, g1, ms], s8t[:, g1, ns],
                                 start=False, stop=(not R_COMP), perf_mode=DR)
                if R_COMP:
                    nc.tensor.matmul(pm[:], r8t[:, :, ms], w8t[:, g0, ns],
                                     start=False, stop=True, perf_mode=DR)

            def evict(dst, pm, final, force_dve=False):
                nonlocal ev
                if final:
                    # keep the whole final eviction on ACT: one sem chain
                    # into the final SP store, avoiding DVE queue delays
                    nc.scalar.copy(dst, pm[:])
                elif force_dve or ev % 2 != 0:
                    nc.vector.tensor_copy(dst, pm[:])
                else:
                    nc.scalar.copy(dst, pm[:])
                ev += 1

            FM = MT - 1       # final m-tile: its cs0 half is hoisted early
            for c in range(NCHUNK):
                lead = LEAD if c == 0 else 0
                final_c = c == NCHUNK - 1
                otiles = {}
                cs0 = slice(c * CW, c * CW + 512)
                cs1 = slice(c * CW + 512, (c + 1) * CW)
                # nb0-only lead: these groups need just x rows + the first
                # 512-col w/s pieces, so PE saturates early
                for m in range(lead):
                    ms = slice(m * 128, (m + 1) * 128)
                    otiles[m] = ostage.tile([128, CW], F16, name="ot", tag="ot")
                    pm = mmps.tile([128, 512], F32, name="pm", tag="pm")
                    group(pm, ms, cs0)
                    evict(otiles[m][:, :512], pm, False)
                if final_c:
                    # hoist the final tile's cs0 half to the chunk start so
                    # its store drains long before the tail
                    fs = slice(FM * 128, (FM + 1) * 128)
                    otiles[FM] = ostage.tile([128, CW], F16, name="ot",
                                             tag="ot")
                    pm = mmps.tile([128, 512], F32, name="pm", tag="pm")
                    group(pm, fs, cs0)
                    evict(otiles[FM][:, :512], pm, False)
                    nc.sync.dma_start(outd[fs, cs0], otiles[FM][:, :512])
                for m in range(MT):
                    ms = slice(m * 128, (m + 1) * 128)
                    last = final_c and m == MT - 1
                    if m >= lead and not (final_c and m == FM):
                        otiles[m] = ostage.tile([128, CW], F16,
                                                name="ot", tag="ot")
                        pm = mmps.tile([128, 512], F32, name="pm", tag="pm")
                        group(pm, ms, cs0)
                        evict(otiles[m][:, :512], pm, False)
                    pm = mmps.tile([128, 512], F32, name="pm", tag="pm")
                    group(pm, ms, cs1)
                    evict(otiles[m][:, 512:], pm, last)
                    if last:
                        # same-engine issue right behind the ACT eviction
                        nc.scalar.dma_start(outd[ms, cs1], otiles[m][:, 512:])
                    else:
                        nc.sync.dma_start(
                            outd[ms, c * CW:(c + 1) * CW], otiles[m][:])
    nc.compile()
    return nc


def _q8(a):
    return np.clip(a, -240.0, 240.0).astype(E4)


def _pack(t8):
    """[rows, 512] fp8 -> [128 (d'), 4 (kgroup*2+pair), rows] with
    d = (slot//2)*256 + (slot%2)*128 + d', matching the DoubleRow layout."""
    a = np.ascontiguousarray(t8.T).reshape(2, 2, 128, t8.shape[0])
    return np.ascontiguousarray(a.transpose(2, 0, 1, 3).reshape(128, 4, -1))


def kernel(x: np.ndarray, weights: np.ndarray) -> np.ndarray:
    from concourse.bass_utils import run_bass_kernel_spmd

    if "nc" not in _cached:
        _cached["nc"] = _build()
    nc = _cached["nc"]

    x = np.ascontiguousarray(x, dtype=np.float32)
    w = np.ascontiguousarray(weights, dtype=np.float32)
    xn = x * (SCALE / np.maximum(np.linalg.norm(x, axis=1, keepdims=True), 1e-8))
    wn = w * (SCALE / np.maximum(np.linalg.norm(w, axis=1, keepdims=True), 1e-8))

    x8 = _q8(xn)
    w8 = _q8(wn)
    s8 = _q8(wn - w8.astype(np.float32))
    xp = _pack(x8)
    wp = _pack(w8)
    sp = _pack(s8)
    if R_COMP:
        rp = _pack(_q8(xn - x8.astype(np.float32)))[:, :2]

    in_maps = []
    for i in range(NCORES):
        r, cgrid = divmod(i, GC)
        bs = slice(r * BS, (r + 1) * BS)
        cs = slice(cgrid * NW, (cgrid + 1) * NW)
        m = {
            "x8": np.ascontiguousarray(xp[:, :, bs]),
            "w8": np.ascontiguousarray(wp[:, :, cs]),
            "s8": np.ascontiguousarray(sp[:, :, cs]),
        }
        if R_COMP:
            m["r8"] = np.ascontiguousarray(rp[:, :, bs])
        in_maps.append(m)

    res = run_bass_kernel_spmd(nc, in_maps, list(range(NCORES)))
    out = np.empty((B, N), dtype=np.float32)
    inv = np.float32(1.0 / (SCALE * SCALE))
    for i in range(NCORES):
        r, cgrid = divmod(i, GC)
        out[r * BS:(r + 1) * BS, cgrid * NW:(cgrid + 1) * NW] = \
            res.results[i]["out"].astype(np.float32) * inv
    return out


# revision 60
# speedup vs baseline: 1.0018x; 1.0018x over previous
"""Cosine-similarity kernel (x[16384,512] vs weights[4096,512] -> [16384,4096])
on 8 Trainium2 NeuronCores, sharded as a 4x2 grid (4-way batch x 2-way N).

Strategy: cos(x, w) = (x/|x|)·(w/|w|) is a normalized GEMM.  All cheap
O(B*D) prep runs on host: normalize, scale by S, quantize to TRN fp8 e4m3,
and pre-pack transposed k-tile-paired layouts.  The device does only the
O(B*N*D) GEMM as DoubleRow fp8 matmuls (2 k-rows per partition), PSUM
accumulation, fp16 eviction, and big contiguous DMAs.

Precision: the w-side is sent as a two-term fp8 expansion (w8 + s8), so the
device computes x8·(w8+s8) [+ optionally r8·w8 over half of K], keeping
max|err|/absmax(ref) at 1.90e-2 [1.38e-2], under the 2e-2 gate on the fixed
problem seed (device arithmetic reproduces the numpy emulation to ~1e-6).

Layouts: both k-groups are packed into one tensor per operand
([128, 4, rows]: slot = kgroup*2 + pair) so each input DMA piece is >=728ns
and the SP-sequencer issue cadence (565ns/DMA) never starves the DMA pipe.
"""
import numpy as np
import ml_dtypes

B, D, N = 16384, 512, 4096
GR, GC = 4, 2             # grid: GR batch shards x GC column shards
NCORES = GR * GC
BS = B // GR              # 4096 batch rows per core
NW = N // GC              # 2048 output cols per core
MT = BS // 128            # 32 m-tiles
NB = NW // 512            # 4 psum column blocks
NCHUNK = 2                # n-column chunks of 1024 for DMA/compute overlap
SCALE = 128.0             # fp8 dynamic-range scale; out = psum / SCALE^2
R_COMP = 0                # 1: add r8·w8 over K/2 (rel~1.38e-2), 0: rel~1.90e-2
WARMUP = 63               # PE p-state warmup matmuls during input DMA window

E4 = ml_dtypes.float8_e4m3  # IEEE-style e4m3, max normal 240 == TRN FP8_EXP4

_cached = {}


def _build():
    import concourse.bass as bass
    import concourse.mybir as mybir
    import concourse.tile as tile
    from concourse import bacc

    F32, F16, F8 = mybir.dt.float32, mybir.dt.float16, mybir.dt.float8e4
    DR = mybir.MatmulPerfMode.DoubleRow
    CW = NW // NCHUNK     # 1024 cols per chunk

    nc = bacc.Bacc(None, target_bir_lowering=False)
    x8d = nc.dram_tensor("x8", [128, 4, BS], F8, kind="ExternalInput")
    w8d = nc.dram_tensor("w8", [128, 4, NW], F8, kind="ExternalInput")
    s8d = nc.dram_tensor("s8", [128, 4, NW], F8, kind="ExternalInput")
    r8d = (nc.dram_tensor("r8", [128, 2, BS], F8, kind="ExternalInput")
           if R_COMP else None)
    outd = nc.dram_tensor("out", [BS, NW], F16, kind="ExternalOutput")

    with tile.TileContext(nc) as tc:
        with (
            tc.tile_pool(name="ops", bufs=1) as ops,
            tc.tile_pool(name="ostage", bufs=34) as ostage,
            tc.tile_pool(name="mmps", bufs=8, space="PSUM") as mmps,
        ):
            x8t = ops.tile([128, 4, BS], F8, name="x8t")
            w8t = ops.tile([128, 4, NW], F8, name="w8t")
            s8t = ops.tile([128, 4, NW], F8, name="s8t")
            r8t = ops.tile([128, 2, BS], F8, name="r8t") if R_COMP else None
            zpad = ops.tile([128, 2, 128], F8, name="zpad")
            wps = mmps.tile([128, 512], F32, name="wps", tag="pm")

            # Warm up the PE p-state during the input-DMA window: dummy
            # DoubleRow matmuls on a zeroed tile so the real matmuls start at
            # the full 2.4 GHz clock instead of paying the 3us ramp.  The
            # memset is tiny so the PE busy-clock starts early enough that
            # even the first real matmuls run at full speed.
            nc.vector.memset(zpad[:], 0)
            for _ in range(WARMUP):
                nc.tensor.matmul(wps[:, :128], zpad[:], zpad[:],
                                 start=True, stop=True, perf_mode=DR)

            # Input supply matched to PE consumption: x rows 0:512 + w/s
            # chunk-0 cols, then ascending x row ranges, then chunk 1.
            # Every piece is >=728ns of transfer so the 565ns/instr SP
            # sequencer issue rate keeps the DMA pipe saturated.
            h0, h1 = slice(0, 512), slice(512, CW)
            nc.sync.dma_start(x8t[:, :, :512], x8d[:, :, :512])
            nc.sync.dma_start(w8t[:, :, h0], w8d[:, :, h0])
            nc.sync.dma_start(s8t[:, :, h0], s8d[:, :, h0])
            nc.sync.dma_start(x8t[:, :, 512:1024], x8d[:, :, 512:1024])
            nc.sync.dma_start(w8t[:, :, h1], w8d[:, :, h1])
            nc.sync.dma_start(s8t[:, :, h1], s8d[:, :, h1])
            for rs in (slice(1024, 2048), slice(2048, BS)):
                nc.sync.dma_start(x8t[:, :, rs], x8d[:, :, rs])
            if R_COMP:
                nc.sync.dma_start(r8t[:], r8d[:])
            for c in range(1, NCHUNK):
                cs = slice(c * CW, (c + 1) * CW)
                nc.sync.dma_start(w8t[:, :, cs], w8d[:, :, cs])
                nc.sync.dma_start(s8t[:, :, cs], s8d[:, :, cs])

            g0, g1 = slice(0, 2), slice(2, 4)
            ev = 0
            LEAD = 4          # nb0-only lead tiles: saturates PE on 3 DMAs

            def group(pm, ms, ns):
                nc.tensor.matmul(pm[:], x8t[:, g0, ms], w8t[:, g0, ns],
                                 start=True, stop=False, perf_mode=DR)
                nc.tensor.matmul(pm[:], x8t[:, g1, ms], w8t[:, g1, ns],
                                 start=False, stop=False, perf_mode=DR)
                nc.tensor.matmul(pm[:], x8t[:, g0, ms], s8t[:, g0, ns],
                                 start=False, stop=False, perf_mode=DR)
                nc.tensor.matmul(pm[:], x8t[:, g1, ms], s8t[:, g1, ns],
                                 start=False, stop=(not R_COMP), perf_mode=DR)
                if R_COMP:
                    nc.tensor.matmul(pm[:], r8t[:, :, ms], w8t[:, g0, ns],
                                     start=False, stop=True, perf_mode=DR)

            def evict(dst, pm, final, force_dve=False):
                nonlocal ev
                if final:
                    # keep the whole final eviction on ACT: one sem chain
                    # into the final SP store, avoiding DVE queue delays
                    nc.scalar.copy(dst, pm[:])
                elif force_dve or ev % 2 != 0:
                    nc.vector.tensor_copy(dst, pm[:])
                else:
                    nc.scalar.copy(dst, pm[:])
                ev += 1

            FM = MT - 1       # final m-tile: its cs0 half is hoisted early
            for c in range(NCHUNK):
                lead = LEAD if c == 0 else 0
                final_c = c == NCHUNK - 1
                otiles = {}
                cs0 = slice(c * CW, c * CW + 512)
                cs1 = slice(c * CW + 512, (c + 1) * CW)
                # nb0-only lead: these groups need just x rows + the first
                # 512-col w/s pieces, so PE saturates early
                for m in range(lead):
                    ms = slice(m * 128, (m + 1) * 128)
                    otiles[m] = ostage.tile([128, CW], F16, name="ot", tag="ot")
                    pm = mmps.tile([128, 512], F32, name="pm", tag="pm")
                    group(pm, ms, cs0)
                    evict(otiles[m][:, :512], pm, False)
                if final_c:
                    # hoist the final tile's cs0 half to the chunk start so
                    # its store drains long before the tail
                    fs = slice(FM * 128, (FM + 1) * 128)
                    otiles[FM] = ostage.tile([128, CW], F16, name="ot",
                                             tag="ot")
                    pm = mmps.tile([128, 512], F32, name="pm", tag="pm")
                    group(pm, fs, cs0)
                    evict(otiles[FM][:, :512], pm, False)
                    nc.sync.dma_start(outd[fs, cs0], otiles[FM][:, :512])
                for m in range(MT):
                    ms = slice(m * 128, (m + 1) * 128)
                    last = final_c and m == MT - 1
                    if m >= lead and not (final_c and m == FM):
                        otiles[m] = ostage.tile([128, CW], F16,
                                                name="ot", tag="ot")
                        pm = mmps.tile([128, 512], F32, name="pm", tag="pm")
                        group(pm, ms, cs0)
                        evict(otiles[m][:, :512], pm, False)
                    pm = mmps.tile([128, 512], F32, name="pm", tag="pm")
                    group(pm, ms, cs1)
                    evict(otiles[m][:, 512:], pm, last)
                    if last:
                        # same-engine issue right behind the ACT eviction
                        nc.scalar.dma_start(outd[ms, cs1], otiles[m][:, 512:])
                    else:
                        nc.sync.dma_start(
                            outd[ms, c * CW:(c + 1) * CW], otiles[m][:])
    nc.compile()
    return nc


def _q8(a):
    return np.clip(a, -240.0, 240.0).astype(E4)


def _pack(t8):
    """[rows, 512] fp8 -> [128 (d'), 4 (kgroup*2+pair), rows] with
    d = (slot//2)*256 + (slot%2)*128 + d', matching the DoubleRow layout."""
    a = np.ascontiguousarray(t8.T).reshape(2, 2, 128, t8.shape[0])
    return np.ascontiguousarray(a.transpose(2, 0, 1, 3).reshape(128, 4, -1))


def kernel(x: np.ndarray, weights: np.ndarray) -> np.ndarray:
    from concourse.bass_utils import run_bass_kernel_spmd

    if "nc" not in _cached:
        _cached["nc"] = _build()
    nc = _cached["nc"]

    x = np.ascontiguousarray(x, dtype=np.float32)
    w = np.ascontiguousarray(weights, dtype=np.float32)
    xn = x * (SCALE / np.maximum(np.linalg.norm(x, axis=1, keepdims=True), 1e-8))
    wn = w * (SCALE / np.maximum(np.linalg.norm(w, axis=1, keepdims=True), 1e-8))

    x8 = _q8(xn)
    w8 = _q8(wn)
    s8 = _q8(wn - w8.astype(np.float32))
    xp = _pack(x8)
    wp = _pack(w8)
    sp = _pack(s8)
    if R_COMP:
        rp = _pack(_q8(xn - x8.astype(np.float32)))[:, :2]

    in_maps = []
    for i in range(NCORES):
        r, cgrid = divmod(i, GC)
        bs = slice(r * BS, (r + 1) * BS)
        cs = slice(cgrid * NW, (cgrid + 1) * NW)
        m = {
            "x8": np.ascontiguousarray(xp[:, :, bs]),
            "w8": np.ascontiguousarray(wp[:, :, cs]),
            "s8": np.ascontiguousarray(sp[:, :, cs]),
        }
        if R_COMP:
            m["r8"] = np.ascontiguousarray(rp[:, :, bs])
        in_maps.append(m)

    res = run_bass_kernel_spmd(nc, in_maps, list(range(NCORES)))
    out = np.empty((B, N), dtype=np.float32)
    inv = np.float32(1.0 / (SCALE * SCALE))
    for i in range(NCORES):
        r, cgrid = divmod(i, GC)
        out[r * BS:(r + 1) * BS, cgrid * NW:(cgrid + 1) * NW] = \
            res.results[i]["out"].astype(np.float32) * inv
    return out


# revision 61
# speedup vs baseline: 1.0031x; 1.0012x over previous
"""Cosine-similarity kernel (x[16384,512] vs weights[4096,512] -> [16384,4096])
on 8 Trainium2 NeuronCores, sharded as a 4x2 grid (4-way batch x 2-way N).

Strategy: cos(x, w) = (x/|x|)·(w/|w|) is a normalized GEMM.  All cheap
O(B*D) prep runs on host: normalize, scale by S, quantize to TRN fp8 e4m3,
and pre-pack transposed k-tile-paired layouts.  The device does only the
O(B*N*D) GEMM as DoubleRow fp8 matmuls (2 k-rows per partition), PSUM
accumulation, fp16 eviction, and big contiguous DMAs.

Precision: the w-side is sent as a two-term fp8 expansion (w8 + s8), so the
device computes x8·(w8+s8) [+ optionally r8·w8 over half of K], keeping
max|err|/absmax(ref) at 1.90e-2 [1.38e-2], under the 2e-2 gate on the fixed
problem seed (device arithmetic reproduces the numpy emulation to ~1e-6).

Layouts: both k-groups are packed into one tensor per operand
([128, 4, rows]: slot = kgroup*2 + pair) so each input DMA piece is >=728ns
and the SP-sequencer issue cadence (565ns/DMA) never starves the DMA pipe.
"""
import numpy as np
import ml_dtypes

B, D, N = 16384, 512, 4096
GR, GC = 4, 2             # grid: GR batch shards x GC column shards
NCORES = GR * GC
BS = B // GR              # 4096 batch rows per core
NW = N // GC              # 2048 output cols per core
MT = BS // 128            # 32 m-tiles
NB = NW // 512            # 4 psum column blocks
NCHUNK = 2                # n-column chunks of 1024 for DMA/compute overlap
SCALE = 128.0             # fp8 dynamic-range scale; out = psum / SCALE^2
R_COMP = 0                # 1: add r8·w8 over K/2 (rel~1.38e-2), 0: rel~1.90e-2
WARMUP = 63               # PE p-state warmup matmuls during input DMA window

E4 = ml_dtypes.float8_e4m3  # IEEE-style e4m3, max normal 240 == TRN FP8_EXP4

_cached = {}


def _build():
    import concourse.bass as bass
    import concourse.mybir as mybir
    import concourse.tile as tile
    from concourse import bacc

    F32, F16, F8 = mybir.dt.float32, mybir.dt.float16, mybir.dt.float8e4
    DR = mybir.MatmulPerfMode.DoubleRow
    CW = NW // NCHUNK     # 1024 cols per chunk

    nc = bacc.Bacc(None, target_bir_lowering=False)
    x8d = nc.dram_tensor("x8", [128, 4, BS], F8, kind="ExternalInput")
    w8d = nc.dram_tensor("w8", [128, 4, NW], F8, kind="ExternalInput")
    s8d = nc.dram_tensor("s8", [128, 4, NW], F8, kind="ExternalInput")
    r8d = (nc.dram_tensor("r8", [128, 2, BS], F8, kind="ExternalInput")
           if R_COMP else None)
    outd = nc.dram_tensor("out", [BS, NW], F16, kind="ExternalOutput")

    with tile.TileContext(nc) as tc:
        with (
            tc.tile_pool(name="ops", bufs=1) as ops,
            tc.tile_pool(name="ostage", bufs=64) as ostage,
            tc.tile_pool(name="mmps", bufs=8, space="PSUM") as mmps,
        ):
            x8t = ops.tile([128, 4, BS], F8, name="x8t")
            w8t = ops.tile([128, 4, NW], F8, name="w8t")
            s8t = ops.tile([128, 4, NW], F8, name="s8t")
            r8t = ops.tile([128, 2, BS], F8, name="r8t") if R_COMP else None
            zpad = ops.tile([128, 2, 128], F8, name="zpad")
            wps = mmps.tile([128, 512], F32, name="wps", tag="pm")

            # Warm up the PE p-state during the input-DMA window: dummy
            # DoubleRow matmuls on a zeroed tile so the real matmuls start at
            # the full 2.4 GHz clock instead of paying the 3us ramp.  The
            # memset is tiny so the PE busy-clock starts early enough that
            # even the first real matmuls run at full speed.
            nc.vector.memset(zpad[:], 0)
            for _ in range(WARMUP):
                nc.tensor.matmul(wps[:, :128], zpad[:], zpad[:],
                                 start=True, stop=True, perf_mode=DR)

            # Input supply matched to PE consumption: x rows 0:512 + w/s
            # chunk-0 cols, then ascending x row ranges, then chunk 1.
            # Every piece is >=728ns of transfer so the 565ns/instr SP
            # sequencer issue rate keeps the DMA pipe saturated.
            h0, h1 = slice(0, 512), slice(512, CW)
            nc.sync.dma_start(x8t[:, :, :512], x8d[:, :, :512])
            nc.sync.dma_start(w8t[:, :, h0], w8d[:, :, h0])
            nc.sync.dma_start(s8t[:, :, h0], s8d[:, :, h0])
            nc.sync.dma_start(x8t[:, :, 512:1024], x8d[:, :, 512:1024])
            nc.sync.dma_start(w8t[:, :, h1], w8d[:, :, h1])
            nc.sync.dma_start(s8t[:, :, h1], s8d[:, :, h1])
            for rs in (slice(1024, 2048), slice(2048, BS)):
                nc.sync.dma_start(x8t[:, :, rs], x8d[:, :, rs])
            if R_COMP:
                nc.sync.dma_start(r8t[:], r8d[:])
            for c in range(1, NCHUNK):
                cs = slice(c * CW, (c + 1) * CW)
                nc.sync.dma_start(w8t[:, :, cs], w8d[:, :, cs])
                nc.sync.dma_start(s8t[:, :, cs], s8d[:, :, cs])

            g0, g1 = slice(0, 2), slice(2, 4)
            ev = 0
            LEAD = 4          # nb0-only lead tiles: saturates PE on 3 DMAs

            def group(pm, ms, ns):
                nc.tensor.matmul(pm[:], x8t[:, g0, ms], w8t[:, g0, ns],
                                 start=True, stop=False, perf_mode=DR)
                nc.tensor.matmul(pm[:], x8t[:, g1, ms], w8t[:, g1, ns],
                                 start=False, stop=False, perf_mode=DR)
                nc.tensor.matmul(pm[:], x8t[:, g0, ms], s8t[:, g0, ns],
                                 start=False, stop=False, perf_mode=DR)
                nc.tensor.matmul(pm[:], x8t[:, g1, ms], s8t[:, g1, ns],
                                 start=False, stop=(not R_COMP), perf_mode=DR)
                if R_COMP:
                    nc.tensor.matmul(pm[:], r8t[:, :, ms], w8t[:, g0, ns],
                                     start=False, stop=True, perf_mode=DR)

            def evict(dst, pm, final, force_dve=False):
                nonlocal ev
                if final:
                    # keep the whole final eviction on ACT: one sem chain
                    # into the final SP store, avoiding DVE queue delays
                    nc.scalar.copy(dst, pm[:])
                elif force_dve or ev % 2 != 0:
                    nc.vector.tensor_copy(dst, pm[:])
                else:
                    nc.scalar.copy(dst, pm[:])
                ev += 1

            FM = MT - 1       # final m-tile: its cs0 half is hoisted early
            for c in range(NCHUNK):
                lead = LEAD if c == 0 else 0
                final_c = c == NCHUNK - 1
                otiles = {}
                cs0 = slice(c * CW, c * CW + 512)
                cs1 = slice(c * CW + 512, (c + 1) * CW)
                # nb0-only lead: these groups need just x rows + the first
                # 512-col w/s pieces, so PE saturates early
                for m in range(lead):
                    ms = slice(m * 128, (m + 1) * 128)
                    otiles[m] = ostage.tile([128, CW], F16, name="ot", tag="ot")
                    pm = mmps.tile([128, 512], F32, name="pm", tag="pm")
                    group(pm, ms, cs0)
                    evict(otiles[m][:, :512], pm, False)
                if final_c:
                    # hoist the final tile's cs0 half to the chunk start so
                    # its store drains long before the tail
                    fs = slice(FM * 128, (FM + 1) * 128)
                    otiles[FM] = ostage.tile([128, CW], F16, name="ot",
                                             tag="ot")
                    pm = mmps.tile([128, 512], F32, name="pm", tag="pm")
                    group(pm, fs, cs0)
                    evict(otiles[FM][:, :512], pm, False)
                    nc.sync.dma_start(outd[fs, cs0], otiles[FM][:, :512])
                for m in range(MT):
                    ms = slice(m * 128, (m + 1) * 128)
                    last = final_c and m == MT - 1
                    if m >= lead and not (final_c and m == FM):
                        otiles[m] = ostage.tile([128, CW], F16,
                                                name="ot", tag="ot")
                        pm = mmps.tile([128, 512], F32, name="pm", tag="pm")
                        group(pm, ms, cs0)
                        evict(otiles[m][:, :512], pm, False)
                    pm = mmps.tile([128, 512], F32, name="pm", tag="pm")
                    group(pm, ms, cs1)
                    evict(otiles[m][:, 512:], pm, last)
                    if last:
                        # same-engine issue right behind the ACT eviction
                        nc.scalar.dma_start(outd[ms, cs1], otiles[m][:, 512:])
                    else:
                        nc.sync.dma_start(
                            outd[ms, c * CW:(c + 1) * CW], otiles[m][:])
    nc.compile()
    return nc


def _q8(a):
    return np.clip(a, -240.0, 240.0).astype(E4)


def _pack(t8):
    """[rows, 512] fp8 -> [128 (d'), 4 (kgroup*2+pair), rows] with
    d = (slot//2)*256 + (slot%2)*128 + d', matching the DoubleRow layout."""
    a = np.ascontiguousarray(t8.T).reshape(2, 2, 128, t8.shape[0])
    return np.ascontiguousarray(a.transpose(2, 0, 1, 3).reshape(128, 4, -1))


def kernel(x: np.ndarray, weights: np.ndarray) -> np.ndarray:
    from concourse.bass_utils import run_bass_kernel_spmd

    if "nc" not in _cached:
        _cached["nc"] = _build()
    nc = _cached["nc"]

    x = np.ascontiguousarray(x, dtype=np.float32)
    w = np.ascontiguousarray(weights, dtype=np.float32)
    xn = x * (SCALE / np.maximum(np.linalg.norm(x, axis=1, keepdims=True), 1e-8))
    wn = w * (SCALE / np.maximum(np.linalg.norm(w, axis=1, keepdims=True), 1e-8))

    x8 = _q8(xn)
    w8 = _q8(wn)
    s8 = _q8(wn - w8.astype(np.float32))
    xp = _pack(x8)
    wp = _pack(w8)
    sp = _pack(s8)
    if R_COMP:
        rp = _pack(_q8(xn - x8.astype(np.float32)))[:, :2]

    in_maps = []
    for i in range(NCORES):
        r, cgrid = divmod(i, GC)
        bs = slice(r * BS, (r + 1) * BS)
        cs = slice(cgrid * NW, (cgrid + 1) * NW)
        m = {
            "x8": np.ascontiguousarray(xp[:, :, bs]),
            "w8": np.ascontiguousarray(wp[:, :, cs]),
            "s8": np.ascontiguousarray(sp[:, :, cs]),
        }
        if R_COMP:
            m["r8"] = np.ascontiguousarray(rp[:, :, bs])
        in_maps.append(m)

    res = run_bass_kernel_spmd(nc, in_maps, list(range(NCORES)))
    out = np.empty((B, N), dtype=np.float32)
    inv = np.float32(1.0 / (SCALE * SCALE))
    for i in range(NCORES):
        r, cgrid = divmod(i, GC)
        out[r * BS:(r + 1) * BS, cgrid * NW:(cgrid + 1) * NW] = \
            res.results[i]["out"].astype(np.float32) * inv
    return out
